# revision 9
# baseline (speedup 1.0000x reference)
"""Sequence-parallel dense attention kernel for 8 Trainium2 NeuronCores.

Math (reference):
    h = x @ W1.T + b1                  [N, H]
    q/k/v = h @ W{q,k,v}.T + b{q,k,v}  [N, H]
    A = softmax(q @ k.T / sqrt(H))     [N, N]
    out = (h + A @ v) @ W2.T + b2      [N]

Algebraic restructuring:
  * out[n] = h[n]@w2 + (A_un[n,:]@z)/(A_un[n,:]@1) + c0 with A_un = exp(scores),
    z = v_nobias @ w2, c0 = b_v@w2 + b2.
  * k = x @ (k_w @ lin1_w).T + (k_w@b1 + k_b)  (host-folded weight)
  * z = x @ (lin1_w.T @ v_w.T @ w2) + const    (host-folded, direct from x)

Optimizations vs the 214us baseline:
  * all projections in bf16 (fp32r HIGH-mode matmuls were 2 cyc/col), q/k in
    fp8e4 with DoubleRow score matmuls (contraction 256 in one pass).
    absmax-rel ~3.4e-3 measured (<2e-2 gate).
  * the 8-rank AllGather costs ~50us after its doorbell (ncfw barrier +
    ring steps) no matter the payload.  To hide it, every core also receives
    x rows of shards 0 and 1 and computes their k/z locally (static
    addressing, identical program on all cores); the 16 score groups of
    shards 0-1 run while the collective distributes shards 2-7.
  * z is transposed locally pre-gather and travels as one fp8 row, so the
    post-gather unpack is a single DVE copy.
  * reduce stationary is a [128,2] strided slice of a precomputed tile;
    epilogue reciprocal on the DVE (no ACT table switches).
"""

import numpy as np

N, D, H = 8192, 1024, 256
NC = 8
S = N // NC          # rows per core
NKC = N // 128       # 64 global nk chunks
NREP = 2             # shards replicated on every core
SCALE = 0.0625       # 1/sqrt(256)

_cache = {}


def _build_program():
    import concourse.tile as tile
    from concourse import bacc, mybir
    from concourse.masks import make_identity

    f32 = mybir.dt.float32
    bf16 = mybir.dt.bfloat16
    f8 = mybir.dt.float8e4
    DR = mybir.MatmulPerfMode.DoubleRow
    Ident = mybir.ActivationFunctionType.Identity
    Exp = mybir.ActivationFunctionType.Exp

    nc = bacc.Bacc("TRN2", target_bir_lowering=False, debug=False, num_devices=NC)

    R = NREP
    xT = nc.dram_tensor("xT", [D, S], bf16, kind="ExternalInput").ap()
    xT01 = nc.dram_tensor("xT01", [D, R * S], bf16, kind="ExternalInput").ap()
    w1T = nc.dram_tensor("w1T", [D, H], bf16, kind="ExternalInput").ap()
    wk1T = nc.dram_tensor("wk1T", [D, H], bf16, kind="ExternalInput").ap()
    wqT = nc.dram_tensor("wqT", [H, H], bf16, kind="ExternalInput").ap()
    # packed fp32 constants (per-partition columns):
    #   0-1 b1 | 2-3 bq | 4-5 bkk=k_w@b1+k_b | 6 row0: c0=v_b@w2+b2 | 7 row0: zc0
    cpk = nc.dram_tensor("cpk", [128, 8], f32, kind="ExternalInput").ap()
    # bf16 stationaries: cols 0-1 w2 (resid)
    cpb = nc.dram_tensor("cpb", [128, 2], bf16, kind="ExternalInput").ap()
    # zw = lin1_w.T @ wv2 packed per d-chunk: col 2*dc = zw chunk, col 2*dc+1 = 0
    zwp = nc.dram_tensor("zwp", [128, 16], bf16, kind="ExternalInput").ap()
    out_d = nc.dram_tensor("out", [1, S], f32, kind="ExternalOutput").ap()

    cc_in = nc.dram_tensor("cc_in", [H + 1, S], f8).ap()
    cc_out = nc.dram_tensor("cc_out", [(H + 1) * NC, S], f8, addr_space="Shared").ap()

    with tile.TileContext(nc) as tc:
        with (
            tc.tile_pool(name="consts", bufs=1) as consts,
            tc.tile_pool(name="xpool", bufs=8) as xpool,
            tc.tile_pool(name="xp2", bufs=8) as xp2,
            tc.tile_pool(name="work", bufs=1) as work,
            tc.tile_pool(name="small", bufs=2) as small,
            tc.tile_pool(name="expp", bufs=11) as expp,
            tc.tile_pool(name="stp", bufs=3, space="PSUM") as stp,
            tc.tile_pool(name="redp", bufs=1, space="PSUM") as redp,
        ):
            # ---- interleaved chunk loads: PE can start after the first chunk ----
            w1sb = consts.tile([128, 8, H], bf16)
            wk1sb = consts.tile([128, 8, H], bf16)
            w1c = w1T.rearrange("(c p) h -> p c h", p=128)
            wk1c = wk1T.rearrange("(c p) h -> p c h", p=128)
            xts = []
            for dc in range(8):
                nc.sync.dma_start(out=wk1sb[:, dc, :], in_=wk1c[:, dc, :])
                xt = xpool.tile([128, S], bf16, tag="xt")
                nc.sync.dma_start(out=xt, in_=xT[dc * 128:(dc + 1) * 128, :])
                xts.append(xt)
            cpack = consts.tile([128, 8], f32)
            nc.sync.dma_start(out=cpack, in_=cpk)
            cpbb = consts.tile([128, 2], bf16)
            nc.sync.dma_start(out=cpbb, in_=cpb)
            zwsb = consts.tile([128, 16], bf16)
            nc.sync.dma_start(out=zwsb, in_=zwp)
            # warm the ACT exp table set before any real activation needs it
            dumm = consts.tile([1, 1], f32)
            nc.vector.memset(dumm, 0.0)
            dumo = consts.tile([1, 1], f32)
            nc.scalar.activation(out=dumo, in_=dumm, func=Exp)

            ident = consts.tile([128, 128], f32)
            make_identity(nc, ident)
            ident1 = consts.tile([1, 1], bf16)
            nc.vector.tensor_copy(out=ident1, in_=ident[0:1, 0:1])

            def kt_z_from_x(xtiles, ncols, ktdst, zrowdst):
                """kt (fp8) + z row (bf16) from x-chunk tiles covering ncols rows."""
                for nt in range(ncols // 512):
                    for hc in range(2):
                        ps = stp.tile([128, 512], f32, tag="st", name="ps")
                        for dc in range(8):
                            nc.tensor.matmul(
                                ps,
                                lhsT=wk1sb[:, dc, hc * 128:(hc + 1) * 128],
                                rhs=xtiles[dc][:, nt * 512:(nt + 1) * 512],
                                start=(dc == 0),
                                stop=(dc == 7),
                            )
                        nc.scalar.activation(
                            out=ktdst[:, hc, nt * 512:(nt + 1) * 512], in_=ps,
                            func=Ident, bias=cpack[:, 4 + hc:4 + hc + 1],
                        )
                    psz = stp.tile([2, 512], f32, tag="st", name="psz")
                    for dc in range(8):
                        nc.tensor.matmul(
                            psz,
                            lhsT=zwsb[:, 2 * dc:2 * dc + 2],
                            rhs=xtiles[dc][:, nt * 512:(nt + 1) * 512],
                            start=(dc == 0),
                            stop=(dc == 7),
                        )
                    nc.scalar.activation(
                        out=zrowdst[:, nt * 512:(nt + 1) * 512], in_=psz[0:1, :],
                        func=Ident, bias=cpack[0:1, 7:8],
                    )

            def z_transpose(zrowsrc, ncols, zt8dst):
                for f in range(ncols // 128):
                    pzt = stp.tile([128, 1], bf16, tag="st", name="pzt")
                    nc.tensor.transpose(
                        out=pzt, in_=zrowsrc[:, f * 128:(f + 1) * 128],
                        identity=ident1[:],
                    )
                    nc.vector.tensor_copy(out=zt8dst[:, f:f + 1], in_=pzt)

            # ---- own shard: ktloc + z -> cc_in (gates the collective doorbell) ----
            ktloc = work.tile([128, 2, S], f8)
            zrowsb = work.tile([1, S], bf16)
            kt_z_from_x(xts, S, ktloc, zrowsb)
            zT8loc = work.tile([128, 8], f8)
            z_transpose(zrowsb, S, zT8loc)

            for hc in range(2):
                nc.sync.dma_start(
                    out=cc_in[hc * 128:(hc + 1) * 128, :], in_=ktloc[:, hc, :]
                )
            nc.sync.dma_start(
                out=cc_in[H:H + 1, :].rearrange("one (p j) -> (one p) j", p=128),
                in_=zT8loc,
            )

            # ---- all-gather k.T + zT (0.26MB per rank, fp8) ----
            nc.gpsimd.collective_compute(
                "AllGather",
                mybir.AluOpType.bypass,
                replica_groups=[list(range(NC))],
                ins=[cc_in[:]],
                outs=[cc_out[:]],
            )

            # ---- replicated shards 0..R-1: local k/z while the gather runs ----
            # (w1/wq first: they gate hT -> qT -> every score group, and are
            # tiny next to the 4MB xT01 stream)
            for dc in range(8):
                nc.sync.dma_start(out=w1sb[:, dc, :], in_=w1c[:, dc, :])
            wqsb = consts.tile([128, 2, H], bf16)
            nc.sync.dma_start(out=wqsb, in_=wqT.rearrange("(c p) h -> p c h", p=128))
            x2 = []
            for dc in range(8):
                xt = xp2.tile([128, R * S], bf16, tag="x2")
                nc.sync.dma_start(out=xt, in_=xT01[dc * 128:(dc + 1) * 128, :])
                x2.append(xt)
            kt01 = work.tile([128, 2, R * S], f8)
            zrow01 = work.tile([1, R * S], bf16)
            kt_z_from_x(x2, R * S, kt01, zrow01)
            zT8l01 = work.tile([128, 8 * R], f8)
            z_transpose(zrow01, R * S, zT8l01)

            # local z columns -> bf16 reduce stationaries (cols: z | ones)
            zrloc = consts.tile([128, 16 * R], bf16)
            nc.vector.memset(zrloc[:, 8 * R:16 * R], 1.0)
            nc.vector.tensor_copy(out=zrloc[:, 0:8 * R], in_=zT8l01)
            zrvloc = zrloc.rearrange("p (two g) -> p g two", two=2)

            # ---- hT, q.T and residual overlap the collective ----
            hTsb = work.tile([128, 2, S], bf16)
            for hc in range(2):
                for nt in range(2):
                    ps = stp.tile([128, 512], f32, tag="st", name="ps")
                    for dc in range(8):
                        nc.tensor.matmul(
                            ps,
                            lhsT=w1sb[:, dc, hc * 128:(hc + 1) * 128],
                            rhs=xts[dc][:, nt * 512:(nt + 1) * 512],
                            start=(dc == 0),
                            stop=(dc == 7),
                        )
                    nc.scalar.activation(
                        out=hTsb[:, hc, nt * 512:(nt + 1) * 512], in_=ps,
                        func=Ident, bias=cpack[:, hc:hc + 1],
                    )
            qTsb = work.tile([128, 2, S], f8)
            for hc in range(2):
                for nt in range(2):
                    ps = stp.tile([128, 512], f32, tag="st", name="ps")
                    for hic in range(2):
                        nc.tensor.matmul(
                            ps,
                            lhsT=wqsb[:, hic, hc * 128:(hc + 1) * 128],
                            rhs=hTsb[:, hic, nt * 512:(nt + 1) * 512],
                            start=(hic == 0),
                            stop=(hic == 1),
                        )
                    nc.scalar.activation(
                        out=qTsb[:, hc, nt * 512:(nt + 1) * 512], in_=ps,
                        func=Ident, bias=cpack[:, 2 + hc:2 + hc + 1],
                    )

            residsb = consts.tile([1, S], f32)
            for nt in range(2):
                psr = stp.tile([1, 512], f32, tag="st", name="psr")
                for hic in range(2):
                    nc.tensor.matmul(
                        psr,
                        lhsT=cpbb[:, hic:hic + 1],
                        rhs=hTsb[:, hic, nt * 512:(nt + 1) * 512],
                        start=(hic == 0),
                        stop=(hic == 1),
                    )
                # resid + c0 folded here
                nc.scalar.activation(
                    out=residsb[:, nt * 512:(nt + 1) * 512], in_=psr,
                    func=Ident, bias=cpack[0:1, 6:7],
                )

            # ---- gathered z for ranks R..7 -> reduce stationaries ----
            NG = NC - R                      # gathered ranks
            cc3 = cc_out.rearrange("(r q) j -> r q j", q=H + 1)
            zt8g = work.tile([128, NG, 8], f8)
            for r in range(R, NC):
                nc.sync.dma_start(
                    out=zt8g[:, r - R, :],
                    in_=cc3[r, H:H + 1, :].rearrange("one (p j) -> (one p) j", p=128),
                )
            zrg = consts.tile([128, 16 * NG], bf16)
            nc.vector.memset(zrg[:, 8 * NG:16 * NG], 1.0)
            nc.vector.tensor_copy(
                out=zrg[:, 0:8 * NG], in_=zt8g.rearrange("p r j -> p (r j)")
            )
            zrvg = zrg.rearrange("p (two g) -> p g two", two=2)

            # ---- load gathered k.T as [128, 2(hc), NG*S] ----
            ktg = work.tile([128, 2, NG * S], f8)
            for r in range(R, NC):
                for hc in range(2):
                    nc.sync.dma_start(
                        out=ktg[:, hc, (r - R) * S:(r - R + 1) * S],
                        in_=cc3[r, hc * 128:(hc + 1) * 128, :],
                    )

            # ---- main loop: local groups first, then gathered ----
            psred = [
                redp.tile([2, 512], f32, tag=f"red{nt}", name=f"psred{nt}")
                for nt in range(2)
            ]
            exps = {}
            NLOC = 8 * R

            def kt_zrv(g):
                if g < NLOC:
                    return kt01[:, :, g * 128:(g + 1) * 128], zrvloc[:, g, :]
                gl = g - NLOC
                return ktg[:, :, gl * 128:(gl + 1) * 128], zrvg[:, gl, :]

            def emit_reduce(g):
                e = exps.pop(g)
                _, zrv = kt_zrv(g)
                for nt in range(2):
                    nc.tensor.matmul(
                        psred[nt],
                        lhsT=zrv,
                        rhs=e[:, nt * 512:(nt + 1) * 512],
                        start=(g == 0),
                        stop=(g == NKC - 1),
                    )

            GRP = 8
            for gb in range(0, NKC, GRP):
                for g in range(gb, gb + GRP):
                    ktsl, _ = kt_zrv(g)
                    st = stp.tile([128, 1024], f32, tag="st")
                    for nt in range(2):
                        nc.tensor.matmul(
                            st[:, nt * 512:(nt + 1) * 512],
                            lhsT=ktsl,
                            rhs=qTsb[:, :, nt * 512:(nt + 1) * 512],
                            start=True,
                            stop=True,
                            perf_mode=DR,
                        )
                    e = expp.tile([128, 1024], bf16, tag="expst")
                    nc.scalar.activation(out=e, in_=st, func=Exp, scale=SCALE)
                    exps[g] = e
                if gb > 0:
                    for g in range(gb - GRP, gb):
                        emit_reduce(g)
            for g in range(NKC - GRP, NKC):
                emit_reduce(g)

            # ---- epilogue: out = resid + num/den (den recip on DVE) ----
            outsb = consts.tile([1, S], f32)
            for nt in range(2):
                ndsb = small.tile([2, 512], f32, tag="nd")
                nc.vector.tensor_copy(out=ndsb, in_=psred[nt])
                den0 = small.tile([1, 512], f32, tag="den")
                nc.sync.dma_start(out=den0, in_=ndsb[1:2, :])
                rden = small.tile([1, 512], f32, tag="rden")
                nc.vector.reciprocal_approx_fast(out=rden, in_=den0)
                m = small.tile([1, 512], f32, tag="m")
                nc.vector.tensor_mul(m, ndsb[0:1, :], rden)
                nc.vector.tensor_add(
                    outsb[:, nt * 512:(nt + 1) * 512], m,
                    residsb[:, nt * 512:(nt + 1) * 512],
                )
            nc.sync.dma_start(out=out_d[:], in_=outsb)

    nc.compile()
    return nc


def _get_program():
    if "nc" not in _cache:
        _cache["nc"] = _build_program()
    return _cache["nc"]


def kernel(x, lin1_w, lin1_b, q_w, q_b, k_w, k_b, v_w, v_b, lin2_w, lin2_b):
    import ml_dtypes
    from concourse.bass_utils import run_bass_kernel_spmd

    bf = ml_dtypes.bfloat16
    x = np.asarray(x, dtype=np.float32)
    lin1_w = np.asarray(lin1_w, dtype=np.float32)
    lin1_b = np.asarray(lin1_b, dtype=np.float32)
    q_w = np.asarray(q_w, dtype=np.float32)
    q_b = np.asarray(q_b, dtype=np.float32)
    k_w = np.asarray(k_w, dtype=np.float32)
    k_b = np.asarray(k_b, dtype=np.float32)
    v_w = np.asarray(v_w, dtype=np.float32)
    v_b = np.asarray(v_b, dtype=np.float32)
    lin2_w = np.asarray(lin2_w, dtype=np.float32)
    lin2_b = np.asarray(lin2_b, dtype=np.float32)

    nc = _get_program()

    wk1 = (k_w.astype(np.float64) @ lin1_w.astype(np.float64)).astype(np.float32)
    bkk = (k_w.astype(np.float64) @ lin1_b.astype(np.float64)).astype(np.float32) + k_b
    w2 = lin2_w[0]                                  # [H]
    wv2 = (v_w.T.astype(np.float64) @ w2.astype(np.float64)).astype(np.float32)
    c0 = np.float32(v_b @ w2 + lin2_b[0])
    zw = (lin1_w.T.astype(np.float64) @ wv2.astype(np.float64)).astype(np.float32)
    zc0 = np.float32(wv2.astype(np.float64) @ lin1_b.astype(np.float64))

    cpk = np.zeros((128, 8), dtype=np.float32)
    cpk[:, 0:2] = lin1_b.reshape(2, 128).T
    cpk[:, 2:4] = q_b.reshape(2, 128).T
    cpk[:, 4:6] = bkk.reshape(2, 128).T
    cpk[0, 6] = c0
    cpk[0, 7] = zc0
    cpb = np.zeros((128, 2), dtype=bf)
    cpb[:, 0:2] = w2.reshape(2, 128).T.astype(bf)
    zwp = np.zeros((128, 16), dtype=bf)
    zwp[:, 0::2] = zw.reshape(8, 128).T.astype(bf)

    w1T = np.ascontiguousarray(lin1_w.T).astype(bf)     # [D, H]
    wk1T = np.ascontiguousarray(wk1.T).astype(bf)       # [D, H]
    wqT = np.ascontiguousarray(q_w.T).astype(bf)        # [H, H]
    xT01 = np.ascontiguousarray(x[0:NREP * S, :].T).astype(bf)

    in_maps = []
    for i in range(NC):
        in_maps.append({
            "xT": np.ascontiguousarray(x[i * S:(i + 1) * S, :].T).astype(bf),
            "xT01": xT01,
            "w1T": w1T, "wk1T": wk1T, "wqT": wqT,
            "cpk": cpk, "cpb": cpb, "zwp": zwp,
        })

    res = run_bass_kernel_spmd(nc, in_maps, core_ids=list(range(NC)))
    out = np.concatenate([res.results[i]["out"].reshape(S) for i in range(NC)])
    return out.astype(np.float32)


# revision 12
# speedup vs baseline: 1.1043x; 1.1043x over previous
"""Sequence-parallel dense attention kernel for 8 Trainium2 NeuronCores.

Math (reference):
    h = x @ W1.T + b1                  [N, H]
    q/k/v = h @ W{q,k,v}.T + b{q,k,v}  [N, H]
    A = softmax(q @ k.T / sqrt(H))     [N, N]
    out = (h + A @ v) @ W2.T + b2      [N]

Algebraic restructuring:
  * out[n] = h[n]@w2 + (A_un[n,:]@z)/(A_un[n,:]@1) + c0 with A_un = exp(scores),
    z = v_nobias @ w2, c0 = b_v@w2 + b2.
  * k = x @ (k_w @ lin1_w).T + (k_w@b1 + k_b)  (host-folded weight)
  * z = x @ (lin1_w.T @ v_w.T @ w2) + const    (host-folded, direct from x)

Optimizations vs the 214us baseline:
  * all projections in bf16 (fp32r HIGH-mode matmuls were 2 cyc/col), q/k in
    fp8e4 with DoubleRow score matmuls (contraction 256 in one pass).
    absmax-rel ~3.4e-3 measured (<2e-2 gate).
  * the 8-rank AllGather costs ~50us after its doorbell (ncfw barrier +
    ring steps) no matter the payload.  To hide it, every core also receives
    x rows of shards 0 and 1 and computes their k/z locally (static
    addressing, identical program on all cores); the 16 score groups of
    shards 0-1 run while the collective distributes shards 2-7.
  * z is transposed locally pre-gather and travels as one fp8 row, so the
    post-gather unpack is a single DVE copy.
  * reduce stationary is a [128,2] strided slice of a precomputed tile;
    epilogue reciprocal on the DVE (no ACT table switches).
"""

import numpy as np

N, D, H = 8192, 1024, 256
NC = 8
S = N // NC          # rows per core
NKC = N // 128       # 64 global nk chunks
NREP = 2             # shards replicated on every core
SCALE = 0.0625       # 1/sqrt(256)

_cache = {}


def _build_program():
    import concourse.tile as tile
    from concourse import bacc, mybir
    from concourse.masks import make_identity

    f32 = mybir.dt.float32
    bf16 = mybir.dt.bfloat16
    f8 = mybir.dt.float8e4
    DR = mybir.MatmulPerfMode.DoubleRow
    Ident = mybir.ActivationFunctionType.Identity
    Exp = mybir.ActivationFunctionType.Exp

    nc = bacc.Bacc("TRN2", target_bir_lowering=False, debug=False, num_devices=NC)

    R = NREP
    xT = nc.dram_tensor("xT", [D, S], bf16, kind="ExternalInput").ap()
    xT01 = nc.dram_tensor("xT01", [D, R * S], bf16, kind="ExternalInput").ap()
    w1T = nc.dram_tensor("w1T", [D, H], bf16, kind="ExternalInput").ap()
    wk1T = nc.dram_tensor("wk1T", [D, H], bf16, kind="ExternalInput").ap()
    wqT = nc.dram_tensor("wqT", [H, H], bf16, kind="ExternalInput").ap()
    # packed fp32 constants (per-partition columns):
    #   0-1 b1 | 2-3 bq | 4-5 bkk=k_w@b1+k_b | 6 row0: c0=v_b@w2+b2 | 7 row0: zc0
    cpk = nc.dram_tensor("cpk", [128, 8], f32, kind="ExternalInput").ap()
    # bf16 stationaries: cols 0-1 w2 (resid)
    cpb = nc.dram_tensor("cpb", [128, 2], bf16, kind="ExternalInput").ap()
    # zw = lin1_w.T @ wv2 packed per d-chunk: col 2*dc = zw chunk, col 2*dc+1 = 0
    zwp = nc.dram_tensor("zwp", [128, 16], bf16, kind="ExternalInput").ap()
    out_d = nc.dram_tensor("out", [1, S], f32, kind="ExternalOutput").ap()

    cc_in = nc.dram_tensor("cc_in", [H + 1, S], f8).ap()
    cc_out = nc.dram_tensor("cc_out", [(H + 1) * NC, S], f8, addr_space="Shared").ap()

    with tile.TileContext(nc) as tc:
        with (
            tc.tile_pool(name="consts", bufs=1) as consts,
            tc.tile_pool(name="xpool", bufs=8) as xpool,
            tc.tile_pool(name="xp2", bufs=8) as xp2,
            tc.tile_pool(name="work", bufs=1) as work,
            tc.tile_pool(name="small", bufs=2) as small,
            tc.tile_pool(name="expp", bufs=18) as expp,
            tc.tile_pool(name="stp", bufs=3, space="PSUM") as stp,
            tc.tile_pool(name="redp", bufs=1, space="PSUM") as redp,
        ):
            # ---- interleaved chunk loads: PE can start after the first chunk ----
            w1sb = consts.tile([128, 8, H], bf16)
            wk1sb = consts.tile([128, 8, H], bf16)
            w1c = w1T.rearrange("(c p) h -> p c h", p=128)
            wk1c = wk1T.rearrange("(c p) h -> p c h", p=128)
            xts = []
            for dc in range(8):
                nc.sync.dma_start(out=wk1sb[:, dc, :], in_=wk1c[:, dc, :])
                xt = xpool.tile([128, S], bf16, tag="xt")
                nc.sync.dma_start(out=xt, in_=xT[dc * 128:(dc + 1) * 128, :])
                xts.append(xt)
            cpack = consts.tile([128, 8], f32)
            nc.sync.dma_start(out=cpack, in_=cpk)
            cpbb = consts.tile([128, 2], bf16)
            nc.sync.dma_start(out=cpbb, in_=cpb)
            zwsb = consts.tile([128, 16], bf16)
            nc.sync.dma_start(out=zwsb, in_=zwp)
            # warm the ACT exp table set before any real activation needs it
            dumm = consts.tile([1, 1], f32)
            nc.vector.memset(dumm, 0.0)
            dumo = consts.tile([1, 1], f32)
            nc.scalar.activation(out=dumo, in_=dumm, func=Exp)

            ident = consts.tile([128, 128], f32)
            make_identity(nc, ident)
            ident1 = consts.tile([1, 1], bf16)
            nc.vector.tensor_copy(out=ident1, in_=ident[0:1, 0:1])

            def kt_z_from_x(xtiles, ncols, ktdst, zrowdst):
                """kt (fp8) + z row (bf16) from x-chunk tiles covering ncols rows."""
                for nt in range(ncols // 512):
                    for hc in range(2):
                        ps = stp.tile([128, 512], f32, tag="st", name="ps")
                        for dc in range(8):
                            nc.tensor.matmul(
                                ps,
                                lhsT=wk1sb[:, dc, hc * 128:(hc + 1) * 128],
                                rhs=xtiles[dc][:, nt * 512:(nt + 1) * 512],
                                start=(dc == 0),
                                stop=(dc == 7),
                            )
                        nc.scalar.activation(
                            out=ktdst[:, hc, nt * 512:(nt + 1) * 512], in_=ps,
                            func=Ident, bias=cpack[:, 4 + hc:4 + hc + 1],
                        )
                    psz = stp.tile([2, 512], f32, tag="st", name="psz")
                    for dc in range(8):
                        nc.tensor.matmul(
                            psz,
                            lhsT=zwsb[:, 2 * dc:2 * dc + 2],
                            rhs=xtiles[dc][:, nt * 512:(nt + 1) * 512],
                            start=(dc == 0),
                            stop=(dc == 7),
                        )
                    nc.scalar.activation(
                        out=zrowdst[:, nt * 512:(nt + 1) * 512], in_=psz[0:1, :],
                        func=Ident, bias=cpack[0:1, 7:8],
                    )

            def z_transpose(zrowsrc, ncols, zt8dst):
                for f in range(ncols // 128):
                    pzt = stp.tile([128, 1], bf16, tag="st", name="pzt")
                    nc.tensor.transpose(
                        out=pzt, in_=zrowsrc[:, f * 128:(f + 1) * 128],
                        identity=ident1[:],
                    )
                    nc.vector.tensor_copy(out=zt8dst[:, f:f + 1], in_=pzt)

            # ---- own shard: ktloc + z -> cc_in (gates the collective doorbell) ----
            # high_priority: the scheduler must not interleave kt01/hT work
            # ahead of these -- every cycle of delay here delays the gather.
            with tc.high_priority():
                ktloc = work.tile([128, 2, S], f8)
                zrowsb = work.tile([1, S], bf16)
                kt_z_from_x(xts, S, ktloc, zrowsb)
                zT8loc = work.tile([128, 8], f8)
                z_transpose(zrowsb, S, zT8loc)

                for hc in range(2):
                    nc.sync.dma_start(
                        out=cc_in[hc * 128:(hc + 1) * 128, :], in_=ktloc[:, hc, :]
                    )
                nc.sync.dma_start(
                    out=cc_in[H:H + 1, :].rearrange("one (p j) -> (one p) j", p=128),
                    in_=zT8loc,
                )

            # ---- all-gather k.T + zT (0.26MB per rank, fp8) ----
            nc.gpsimd.collective_compute(
                "AllGather",
                mybir.AluOpType.bypass,
                replica_groups=[list(range(NC))],
                ins=[cc_in[:]],
                outs=[cc_out[:]],
            )

            # ---- replicated shards 0..R-1: local k/z while the gather runs ----
            # (w1/wq first: they gate hT -> qT -> every score group, and are
            # tiny next to the 4MB xT01 stream)
            for dc in range(8):
                nc.sync.dma_start(out=w1sb[:, dc, :], in_=w1c[:, dc, :])
            wqsb = consts.tile([128, 2, H], bf16)
            nc.sync.dma_start(out=wqsb, in_=wqT.rearrange("(c p) h -> p c h", p=128))
            x2 = []
            for dc in range(8):
                xt = xp2.tile([128, R * S], bf16, tag="x2")
                nc.sync.dma_start(out=xt, in_=xT01[dc * 128:(dc + 1) * 128, :])
                x2.append(xt)
            kt01 = work.tile([128, 2, R * S], f8)
            zrow01 = work.tile([1, R * S], bf16)
            kt_z_from_x(x2, R * S, kt01, zrow01)
            zT8l01 = work.tile([128, 8 * R], f8)
            z_transpose(zrow01, R * S, zT8l01)

            # local z columns -> bf16 reduce stationaries (cols: z | ones)
            zrloc = consts.tile([128, 16 * R], bf16)
            nc.vector.memset(zrloc[:, 8 * R:16 * R], 1.0)
            nc.vector.tensor_copy(out=zrloc[:, 0:8 * R], in_=zT8l01)
            zrvloc = zrloc.rearrange("p (two g) -> p g two", two=2)

            # ---- hT, q.T and residual overlap the collective ----
            hTsb = work.tile([128, 2, S], bf16)
            for hc in range(2):
                for nt in range(2):
                    ps = stp.tile([128, 512], f32, tag="st", name="ps")
                    for dc in range(8):
                        nc.tensor.matmul(
                            ps,
                            lhsT=w1sb[:, dc, hc * 128:(hc + 1) * 128],
                            rhs=xts[dc][:, nt * 512:(nt + 1) * 512],
                            start=(dc == 0),
                            stop=(dc == 7),
                        )
                    nc.scalar.activation(
                        out=hTsb[:, hc, nt * 512:(nt + 1) * 512], in_=ps,
                        func=Ident, bias=cpack[:, hc:hc + 1],
                    )
            qTsb = work.tile([128, 2, S], f8)
            for hc in range(2):
                for nt in range(2):
                    ps = stp.tile([128, 512], f32, tag="st", name="ps")
                    for hic in range(2):
                        nc.tensor.matmul(
                            ps,
                            lhsT=wqsb[:, hic, hc * 128:(hc + 1) * 128],
                            rhs=hTsb[:, hic, nt * 512:(nt + 1) * 512],
                            start=(hic == 0),
                            stop=(hic == 1),
                        )
                    nc.scalar.activation(
                        out=qTsb[:, hc, nt * 512:(nt + 1) * 512], in_=ps,
                        func=Ident, bias=cpack[:, 2 + hc:2 + hc + 1],
                    )

            residsb = consts.tile([1, S], f32)
            for nt in range(2):
                psr = stp.tile([1, 512], f32, tag="st", name="psr")
                for hic in range(2):
                    nc.tensor.matmul(
                        psr,
                        lhsT=cpbb[:, hic:hic + 1],
                        rhs=hTsb[:, hic, nt * 512:(nt + 1) * 512],
                        start=(hic == 0),
                        stop=(hic == 1),
                    )
                # resid + c0 folded here
                nc.scalar.activation(
                    out=residsb[:, nt * 512:(nt + 1) * 512], in_=psr,
                    func=Ident, bias=cpack[0:1, 6:7],
                )

            # ---- gathered z for ranks R..7 -> reduce stationaries ----
            NG = NC - R                      # gathered ranks
            cc3 = cc_out.rearrange("(r q) j -> r q j", q=H + 1)
            zt8g = work.tile([128, NG, 8], f8)
            for r in range(R, NC):
                nc.sync.dma_start(
                    out=zt8g[:, r - R, :],
                    in_=cc3[r, H:H + 1, :].rearrange("one (p j) -> (one p) j", p=128),
                )
            zrg = consts.tile([128, 16 * NG], bf16)
            nc.vector.memset(zrg[:, 8 * NG:16 * NG], 1.0)
            nc.vector.tensor_copy(
                out=zrg[:, 0:8 * NG], in_=zt8g.rearrange("p r j -> p (r j)")
            )
            zrvg = zrg.rearrange("p (two g) -> p g two", two=2)

            # ---- load gathered k.T as [128, 2(hc), NG*S] ----
            ktg = work.tile([128, 2, NG * S], f8)
            for r in range(R, NC):
                for hc in range(2):
                    nc.sync.dma_start(
                        out=ktg[:, hc, (r - R) * S:(r - R + 1) * S],
                        in_=cc3[r, hc * 128:(hc + 1) * 128, :],
                    )

            # ---- main loop: local groups first, then gathered ----
            psred = [
                redp.tile([2, 512], f32, tag=f"red{nt}", name=f"psred{nt}")
                for nt in range(2)
            ]
            exps = {}
            NLOC = 8 * R

            def kt_zrv(g):
                if g < NLOC:
                    return kt01[:, :, g * 128:(g + 1) * 128], zrvloc[:, g, :]
                gl = g - NLOC
                return ktg[:, :, gl * 128:(gl + 1) * 128], zrvg[:, gl, :]

            def emit_reduce(g):
                e = exps.pop(g)
                _, zrv = kt_zrv(g)
                for nt in range(2):
                    nc.tensor.matmul(
                        psred[nt],
                        lhsT=zrv,
                        rhs=e[:, nt * 512:(nt + 1) * 512],
                        start=(g == 0),
                        stop=(g == NKC - 1),
                    )

            GRP = 8

            def emit_block(gb):
                for g in range(gb, gb + GRP):
                    ktsl, _ = kt_zrv(g)
                    st = stp.tile([128, 1024], f32, tag="st")
                    for nt in range(2):
                        nc.tensor.matmul(
                            st[:, nt * 512:(nt + 1) * 512],
                            lhsT=ktsl,
                            rhs=qTsb[:, :, nt * 512:(nt + 1) * 512],
                            start=True,
                            stop=True,
                            perf_mode=DR,
                        )
                    e = expp.tile([128, 1024], bf16, tag="expst")
                    nc.scalar.activation(out=e, in_=st, func=Exp, scale=SCALE)
                    exps[g] = e

            # local phase: all reduces drained before any gather-gated score
            # matmul enters the PE queue (FIFO -- a stalled gathered score
            # would trap ready local reduces behind it).
            for gb in range(0, NLOC, GRP):
                emit_block(gb)
                if gb > 0:
                    for g in range(gb - GRP, gb):
                        emit_reduce(g)
            for g in range(NLOC - GRP, NLOC):
                emit_reduce(g)
            # gathered phase
            for gb in range(NLOC, NKC, GRP):
                emit_block(gb)
                if gb > NLOC:
                    for g in range(gb - GRP, gb):
                        emit_reduce(g)
            for g in range(NKC - GRP, NKC):
                emit_reduce(g)

            # ---- epilogue: out = resid + num/den (den recip on DVE) ----
            outsb = consts.tile([1, S], f32)
            for nt in range(2):
                ndsb = small.tile([2, 512], f32, tag="nd")
                nc.vector.tensor_copy(out=ndsb, in_=psred[nt])
                den0 = small.tile([1, 512], f32, tag="den")
                nc.sync.dma_start(out=den0, in_=ndsb[1:2, :])
                rden = small.tile([1, 512], f32, tag="rden")
                nc.vector.reciprocal_approx_fast(out=rden, in_=den0)
                m = small.tile([1, 512], f32, tag="m")
                nc.vector.tensor_mul(m, ndsb[0:1, :], rden)
                nc.vector.tensor_add(
                    outsb[:, nt * 512:(nt + 1) * 512], m,
                    residsb[:, nt * 512:(nt + 1) * 512],
                )
            nc.sync.dma_start(out=out_d[:], in_=outsb)

    nc.compile()
    return nc


def _get_program():
    if "nc" not in _cache:
        _cache["nc"] = _build_program()
    return _cache["nc"]


def kernel(x, lin1_w, lin1_b, q_w, q_b, k_w, k_b, v_w, v_b, lin2_w, lin2_b):
    import ml_dtypes
    from concourse.bass_utils import run_bass_kernel_spmd

    bf = ml_dtypes.bfloat16
    x = np.asarray(x, dtype=np.float32)
    lin1_w = np.asarray(lin1_w, dtype=np.float32)
    lin1_b = np.asarray(lin1_b, dtype=np.float32)
    q_w = np.asarray(q_w, dtype=np.float32)
    q_b = np.asarray(q_b, dtype=np.float32)
    k_w = np.asarray(k_w, dtype=np.float32)
    k_b = np.asarray(k_b, dtype=np.float32)
    v_w = np.asarray(v_w, dtype=np.float32)
    v_b = np.asarray(v_b, dtype=np.float32)
    lin2_w = np.asarray(lin2_w, dtype=np.float32)
    lin2_b = np.asarray(lin2_b, dtype=np.float32)

    nc = _get_program()

    wk1 = (k_w.astype(np.float64) @ lin1_w.astype(np.float64)).astype(np.float32)
    bkk = (k_w.astype(np.float64) @ lin1_b.astype(np.float64)).astype(np.float32) + k_b
    w2 = lin2_w[0]                                  # [H]
    wv2 = (v_w.T.astype(np.float64) @ w2.astype(np.float64)).astype(np.float32)
    c0 = np.float32(v_b @ w2 + lin2_b[0])
    zw = (lin1_w.T.astype(np.float64) @ wv2.astype(np.float64)).astype(np.float32)
    zc0 = np.float32(wv2.astype(np.float64) @ lin1_b.astype(np.float64))

    cpk = np.zeros((128, 8), dtype=np.float32)
    cpk[:, 0:2] = lin1_b.reshape(2, 128).T
    cpk[:, 2:4] = q_b.reshape(2, 128).T
    cpk[:, 4:6] = bkk.reshape(2, 128).T
    cpk[0, 6] = c0
    cpk[0, 7] = zc0
    cpb = np.zeros((128, 2), dtype=bf)
    cpb[:, 0:2] = w2.reshape(2, 128).T.astype(bf)
    zwp = np.zeros((128, 16), dtype=bf)
    zwp[:, 0::2] = zw.reshape(8, 128).T.astype(bf)

    w1T = np.ascontiguousarray(lin1_w.T).astype(bf)     # [D, H]
    wk1T = np.ascontiguousarray(wk1.T).astype(bf)       # [D, H]
    wqT = np.ascontiguousarray(q_w.T).astype(bf)        # [H, H]
    xT01 = np.ascontiguousarray(x[0:NREP * S, :].T).astype(bf)

    in_maps = []
    for i in range(NC):
        in_maps.append({
            "xT": np.ascontiguousarray(x[i * S:(i + 1) * S, :].T).astype(bf),
            "xT01": xT01,
            "w1T": w1T, "wk1T": wk1T, "wqT": wqT,
            "cpk": cpk, "cpb": cpb, "zwp": zwp,
        })

    res = run_bass_kernel_spmd(nc, in_maps, core_ids=list(range(NC)))
    out = np.concatenate([res.results[i]["out"].reshape(S) for i in range(NC)])
    return out.astype(np.float32)
